# revision 2
# baseline (speedup 1.0000x reference)
"""Conv2d 3x3 (stride 1, pad 1) via 1D Winograd F(2,3) along H on TRN2,
data-parallel over batch across 8 NeuronCores.

Full shapes: img [32,128,112,112] f32, weight [256,128,3,3] f32, bias [256] f32
-> out [32,256,112,112] f32.

Per core: 4 images. For each pair of output rows (2p, 2p+1) the 3x3 conv
needs input rows 2p-1..2p+2 (d0..d3).  1D Winograd F(2,3) over ky:
  v0 = d0-d2, v1 = d1+d2, v2 = d2-d1, v3 = d1-d3     (input transform)
  U_j = G-transform of weights along ky (host-side)
  M_j = sum_kx U[j,kx].T @ V_j[shift kx]              (PE, K=C_in=128)
  y_even = M0+M1+M2+bias,  y_odd = M1-M2-M3+bias      (inverse transform)
12 K=128 matmuls per row-pair instead of 18 -> 2/3 the PE row-stream work.

Measured on HW (amplified For_i timing): 431.5us/core vs 475us for the
direct 9-tap formulation.  Config notes: input transform on DVE beat
GPSIMD (Q7 software tensor ops underperform their cost model); the single
ACT M1+bias staging (m1b) beat the two-stage m0s variant; Ldweights dedup
halves PE weight loads (s=0/s=1 matmul pairs share the stationary tile).

PSUM: per pgpair(2 pair-groups)/mt, four [128,2,512] tiles (2 banks each;
s-th half holds M_j of pair-group q+s) -> all 8 banks; inverse drains each
bank just-in-time behind the next pgpair's matmul wave.
"""

import os
import sys

sys.path.insert(0, "/opt/trn_rl_repo")

import numpy as np

N_CORES = 8
N, C_IN, H, W = 32, 128, 112, 112
C_OUT, KH, KW = 256, 3, 3
PER_CORE = N // N_CORES           # 4 images
NPAIR = H // 2                    # 56 row pairs
RPP = 4                           # row-pairs per psum half-tile
NPG = NPAIR // RPP                # 14 pair-groups
PG_HALF = NPG // 2                # 7 pair-groups per image half
MT = C_OUT // 128                 # 2 C_out tiles
WP = W + 2                        # padded columns: 114
HHALF = 28                        # row pairs per half-image
LDR = 58                          # staged rows per half (with halo+pad)

_CACHED = {}

# drop Ldweights instructions whose weights are already loaded (adjacent
# same-weight matmul pairs); 0 = self-loading matmuls as emitted
DEDUP_LDW = os.environ.get("CONV_DEDUP_LDW", "1") == "1"
# input-transform engine: gpsimd | dve | split
V_ENGINE = os.environ.get("CONV_V_ENGINE", "dve")
# inverse-transform staging: m0s (ACT stages M0 copy + M1+bias; frees PSUM
# banks earliest) | m1b (single ACT stage of M1+bias)
INV_SCHEME = os.environ.get("CONV_INV_SCHEME", "m1b")


def _dedup_ldweights(nc):
    """Remove back-to-back redundant PE weight loads.

    The tile scheduler splits every matmul into Ldweights + Matmult.  When
    consecutive matmuls share the same stationary operand (our s=0/s=1
    pg pair), the second Ldweights reloads identical weights; HW pays
    ~53ns each.  Drop the duplicate, preserving its sync waits by moving
    any non-duplicate waits onto the next instruction.
    """
    import concourse.mybir as mybir

    def wait_key(w):
        return (w.sync_type, w.id, w.wait_mode, w.wait_value, str(w.wait_reg))

    def ldw_key(inst):
        return (str(inst.ins[0]), str(inst.perf_mode), str(inst.is_transpose),
                str(inst.tile_position))

    n_removed = 0
    for fn in nc.m.functions:
        for blk in fn.blocks:
            insts = blk.instructions
            rev = {}
            for inst in insts:
                for dep_name, _info in inst.dependency_edges():
                    rev.setdefault(dep_name, []).append(inst)
            keep = []
            last = None          # (inst, key) of last kept Ldweights
            removed_here = set()
            for idx, inst in enumerate(insts):
                op = inst.opcode
                if getattr(inst, "engine", None) != mybir.EngineType.PE:
                    keep.append(inst)
                    continue
                if op == "Ldweights":
                    key = ldw_key(inst)
                    si = inst.sync_info
                    updates = list(si.on_update) if si else []
                    if last is not None and key == last[1] and not updates:
                        kept_si = last[0].sync_info
                        kept_waits = {wait_key(w) for w in
                                      (kept_si.on_wait if kept_si else [])}
                        waits = list(si.on_wait) if si else []
                        extra = [w for w in waits
                                 if wait_key(w) not in kept_waits]
                        if extra and idx + 1 < len(insts):
                            nxt = insts[idx + 1]
                            nsi = nxt.sync_info
                            if nsi is None:
                                nxt.sync_info = mybir.SyncInfo(
                                    on_wait=list(extra), on_update=[])
                            else:
                                nsi.on_wait = list(nsi.on_wait) + list(extra)
                        for dep_inst in rev.get(inst.name, []):
                            dep_inst.remap_dependency_names(
                                {inst.name: last[0].name})
                        removed_here.add(inst.name)
                        n_removed += 1
                        continue
                    last = (inst, key)
                    keep.append(inst)
                elif op in ("Matmult", "MatmultMx"):
                    if getattr(inst, "is_transpose", None):
                        last = None
                    keep.append(inst)
                else:
                    last = None
                    keep.append(inst)
            if removed_here:
                blk.instructions[:] = keep
    return n_removed


def _build(repeat: int = 1):
    import contextlib
    import concourse.tile as tile
    import concourse.mybir as mybir
    from concourse import bacc

    F32 = mybir.dt.float32
    F16 = mybir.dt.float16
    ADD = mybir.AluOpType.add
    SUB = mybir.AluOpType.subtract

    nc = bacc.Bacc("TRN2", target_bir_lowering=False, debug=False)
    img_d = nc.dram_tensor("img", [PER_CORE, C_IN, H, W], F32,
                           kind="ExternalInput").ap()
    # U: [C_in, j, kx, mt, 128] f16 (host-transformed weights)
    u_d = nc.dram_tensor("wt", [C_IN, 4, KW, MT, 128], F16,
                         kind="ExternalInput").ap()
    bias_d = nc.dram_tensor("bias", [128, MT], F32, kind="ExternalInput").ap()
    out_d = nc.dram_tensor("out", [PER_CORE, C_OUT, H, W], F32,
                           kind="ExternalOutput").ap()

    if V_ENGINE == "gpsimd":
        v_engines = ["g", "g", "g", "g"]
    elif V_ENGINE == "dve":
        v_engines = ["v", "v", "v", "v"]
    else:
        v_engines = ["g", "g", "v", "v"]

    with tile.TileContext(nc) as tc:
        with tc.tile_pool(name="const", bufs=1) as const_pool, \
             tc.tile_pool(name="ld", bufs=2) as ld_pool, \
             tc.tile_pool(name="vbuf", bufs=3) as v_pool, \
             tc.tile_pool(name="tmp", bufs=8) as tmp_pool, \
             tc.tile_pool(name="outsb", bufs=2) as out_pool, \
             tc.tile_pool(name="psum", bufs=4, space="PSUM") as psum_pool:

            u_sb = const_pool.tile([C_IN, 4, KW, MT, 128], F16)
            nc.sync.dma_start(u_sb[:], u_d[:])
            bias_sb = const_pool.tile([128, MT], F32)
            nc.sync.dma_start(bias_sb[:], bias_d[:])

            loop_ctx = tc.For_i(0, repeat, 1) if repeat > 1 \
                else contextlib.nullcontext()
            with loop_ctx:
              for i in range(PER_CORE):
                # ---- load + input transform, one half-image per step ----
                vhalves = []
                for h in range(2):
                    ld = ld_pool.tile([C_IN, LDR, W], F32)
                    if h == 0:
                        # slots 0..57 = img rows -1..56 (row -1 zero)
                        nc.vector.memset(ld[:, 0, :], 0.0)
                        nc.sync.dma_start(ld[:, 1:58, :], img_d[i, :, 0:57, :])
                    else:
                        # slots 0..57 = img rows 55..112 (row 112 zero)
                        nc.sync.dma_start(ld[:, 0:57, :],
                                          img_d[i, :, 55:112, :])
                        nc.vector.memset(ld[:, 57, :], 0.0)

                    v = v_pool.tile([C_IN, 4, HHALF, WP], F16)
                    nc.vector.memset(v[:, :, :, 0], 0.0)
                    nc.vector.memset(v[:, :, :, WP - 1], 0.0)
                    d0 = ld[:, 0:55:2, :]
                    d1 = ld[:, 1:56:2, :]
                    d2 = ld[:, 2:57:2, :]
                    d3 = ld[:, 3:58:2, :]
                    vops = [(0, d0, d2, SUB), (1, d1, d2, ADD),
                            (2, d2, d1, SUB), (3, d1, d3, SUB)]
                    for j, a, b, op in vops:
                        eng = nc.gpsimd if v_engines[j] == "g" else nc.vector
                        eng.tensor_tensor(v[:, j, :, 1:1 + W], a, b, op)
                    vhalves.append(v)

                # ---- matmuls + inverse transform per pgpair/mt ----
                for q in range(0, NPG, 2):
                    for mt in range(MT):
                        ps = []
                        for j in range(4):
                            pj = psum_pool.tile([128, 2, 512], F32,
                                                name=f"ps{j}", tag="ps")
                            ps.append(pj)
                        for j in range(4):
                            for kx in range(KW):
                                for s in range(2):
                                    pg = q + s
                                    hh, lp = divmod(pg, PG_HALF)
                                    nc.tensor.matmul(
                                        ps[j][:, s, 0:448],
                                        lhsT=u_sb[:, j, kx, mt, :],
                                        rhs=vhalves[hh][:, j,
                                                        4 * lp:4 * lp + RPP,
                                                        kx:kx + W],
                                        start=(kx == 0), stop=(kx == KW - 1),
                                    )
                        # [128,2,448] -> [128,2,4,112] views of M_j
                        mv = [ps[j][:, :, 0:448].rearrange(
                                  "p s (a w) -> p s a w", a=RPP)
                              for j in range(4)]
                        osb = out_pool.tile([128, 2, RPP, 2, W], F32)
                        m1b = tmp_pool.tile([128, 2, RPP, W], F32, tag="tmp")
                        a0 = tmp_pool.tile([128, 2, RPP, W], F32, tag="tmp")
                        t1 = tmp_pool.tile([128, 2, RPP, W], F32, tag="tmp")
                        if INV_SCHEME == "skel":
                            # DIAGNOSTIC: no DVE inverse; ACT copies M1 pair
                            # into both parities (wrong results, times the
                            # PE + DMA + ACT skeleton)
                            nc.scalar.copy(osb[:, :, :, 0, :], mv[1])
                            nc.scalar.copy(osb[:, :, :, 1, :], mv[2])
                        elif INV_SCHEME == "m0s":
                            # ACT drains M0 (then M1) right as their banks
                            # finish accumulating -> earliest bank reuse
                            m0s = tmp_pool.tile([128, 2, RPP, W], F32, tag="tmp")
                            nc.scalar.copy(m0s[:], mv[0])
                            nc.scalar.add(m1b[:], mv[1],
                                          bias_sb[:, mt:mt + 1])
                            nc.vector.tensor_tensor(a0[:], m0s[:], m1b[:],
                                                    ADD)
                        else:
                            nc.scalar.add(m1b[:], mv[1],
                                          bias_sb[:, mt:mt + 1])
                            nc.vector.tensor_tensor(a0[:], m1b[:], mv[0],
                                                    ADD)
                        nc.vector.tensor_tensor(osb[:, :, :, 0, :],
                                                a0[:], mv[2], ADD)
                        nc.vector.tensor_tensor(t1[:], m1b[:], mv[2], SUB)
                        nc.vector.tensor_tensor(osb[:, :, :, 1, :],
                                                t1[:], mv[3], SUB)
                        for s in range(2):
                            # qACT hwdge queue: keeps the big output stream
                            # off qSP, which carries the input loads
                            nc.scalar.dma_start(
                                out_d[i, mt * 128:(mt + 1) * 128,
                                      8 * (q + s):8 * (q + s) + 8, :],
                                osb[:, s, :, :, :])

    if DEDUP_LDW:
        _dedup_ldweights(nc)
    nc.compile()
    return nc


def _make_runner(nc, donate=True):
    """Cached sharded-jit runner for `nc` on 8 cores (see kernel.py)."""
    import jax
    from jax.sharding import Mesh, PartitionSpec, NamedSharding
    from jax.experimental.shard_map import shard_map
    import concourse.mybir as mybir
    from concourse import bass2jax

    bass2jax.install_neuronx_cc_hook()

    partition_name = nc.partition_id_tensor.name if nc.partition_id_tensor else None
    in_names, out_names, out_avals, zero_outs = [], [], [], []
    for alloc in nc.m.functions[0].allocations:
        if not isinstance(alloc, mybir.MemoryLocationSet):
            continue
        name = alloc.memorylocations[0].name
        if alloc.kind == "ExternalInput":
            if name != partition_name:
                in_names.append(name)
        elif alloc.kind == "ExternalOutput":
            shape = tuple(alloc.tensor_shape)
            dtype = mybir.dt.np(alloc.dtype)
            out_names.append(name)
            out_avals.append(jax.core.ShapedArray(shape, dtype))
            zero_outs.append(np.zeros(shape, dtype))
    n_params = len(in_names)
    n_outs = len(out_avals)
    all_in_names = list(in_names) + list(out_names)
    if partition_name is not None:
        all_in_names.append(partition_name)

    def _body(*args):
        operands = list(args)
        if partition_name is not None:
            operands.append(bass2jax.partition_id_tensor())
        outs = bass2jax._bass_exec_p.bind(
            *operands,
            out_avals=tuple(out_avals),
            in_names=tuple(all_in_names),
            out_names=tuple(out_names),
            lowering_input_output_aliases=(),
            sim_require_finite=True,
            sim_require_nnan=True,
            nc=nc,
        )
        return tuple(outs)

    devices = jax.devices()[:N_CORES]
    mesh = Mesh(np.asarray(devices), ("core",))
    in_specs = (PartitionSpec("core"),) * (n_params + n_outs)
    out_specs = (PartitionSpec("core"),) * len(out_names)
    kwargs = dict(keep_unused=True)
    if donate:
        kwargs["donate_argnums"] = tuple(range(n_params, n_params + n_outs))
    sharded = jax.jit(
        shard_map(_body, mesh=mesh, in_specs=in_specs, out_specs=out_specs,
                  check_rep=False),
        **kwargs)
    sharding = NamedSharding(mesh, PartitionSpec("core"))

    def prep(in_maps, device_put=False):
        concat = [np.concatenate([np.asarray(m[name]) for m in in_maps], axis=0)
                  for name in in_names]
        concat += [np.concatenate([z] * N_CORES, axis=0) for z in zero_outs]
        if device_put:
            concat = [jax.device_put(a, sharding) for a in concat]
        return concat

    def run(args):
        return sharded(*args)

    def to_results(outs):
        results = [dict() for _ in range(N_CORES)]
        for name, arr in zip(out_names, outs):
            arr = np.asarray(arr)
            per = np.split(arr, N_CORES, axis=0)
            for c in range(N_CORES):
                results[c][name] = per[c]
        return results

    return prep, run, to_results


def _host_weights(weight: np.ndarray) -> np.ndarray:
    """[C_out, C_in, 3, 3] f32 -> U [C_in, j(4), kx(3), mt(2), 128] f16."""
    w = weight.astype(np.float64)
    g0 = w[:, :, 0, :]
    g1 = w[:, :, 1, :]
    g2 = w[:, :, 2, :]
    u = np.stack([g0, (g0 + g1 + g2) * 0.5, (g0 - g1 + g2) * 0.5, g2],
                 axis=0)                      # [j, C_out, C_in, kx]
    u = u.transpose(2, 0, 3, 1)               # [C_in, j, kx, C_out]
    u = u.reshape(C_IN, 4, KW, MT, 128)
    return np.ascontiguousarray(u.astype(np.float16))


def kernel(img: np.ndarray, weight: np.ndarray, bias: np.ndarray) -> np.ndarray:
    img = np.ascontiguousarray(np.asarray(img, dtype=np.float32))
    weight = np.ascontiguousarray(np.asarray(weight, dtype=np.float32))
    bias = np.ascontiguousarray(np.asarray(bias, dtype=np.float32))

    u = _host_weights(weight)
    bias2 = np.ascontiguousarray(bias.reshape(MT, 128).T)

    if "nc" not in _CACHED:
        _CACHED["nc"] = _build()
        _CACHED["runner"] = _make_runner(_CACHED["nc"], donate=False)
    prep, run, to_results = _CACHED["runner"]

    shards = img.reshape(N_CORES, PER_CORE, C_IN, H, W)
    in_maps = [{"img": shards[i], "wt": u, "bias": bias2}
               for i in range(N_CORES)]

    outs = run(prep(in_maps))
    results = to_results(outs)
    _CACHED["last_results"] = results
    return np.concatenate([r["out"] for r in results], axis=0)


# revision 3
# speedup vs baseline: 1.0049x; 1.0049x over previous
"""Conv2d 3x3 (stride 1, pad 1) via 1D Winograd F(2,3) along H on TRN2,
data-parallel over batch across 8 NeuronCores.

Full shapes: img [32,128,112,112] f32, weight [256,128,3,3] f32, bias [256] f32
-> out [32,256,112,112] f32.

Per core: 4 images. For each pair of output rows (2p, 2p+1) the 3x3 conv
needs input rows 2p-1..2p+2 (d0..d3).  1D Winograd F(2,3) over ky:
  v0 = d0-d2, v1 = d1+d2, v2 = d2-d1, v3 = d1-d3     (input transform)
  U_j = G-transform of weights along ky (host-side)
  M_j = sum_kx U[j,kx].T @ V_j[shift kx]              (PE, K=C_in=128)
  y_even = M0+M1+M2+bias,  y_odd = M1-M2-M3+bias      (inverse transform)
12 K=128 matmuls per row-pair instead of 18 -> 2/3 the PE row-stream work.

Measured on HW (amplified For_i timing): 416.0us/core vs 475us for the
direct 9-tap formulation.  Config notes: input transform on DVE beat
GPSIMD (Q7 software tensor ops underperform their cost model); the single
ACT M1+bias staging (m1b) beat the two-stage m0s variant; Ldweights dedup
halves PE weight loads (s=0/s=1 matmul pairs share the stationary tile).

PSUM: per pgpair(2 pair-groups)/mt, four [128,2,512] tiles (2 banks each;
s-th half holds M_j of pair-group q+s) -> all 8 banks; inverse drains each
bank just-in-time behind the next pgpair's matmul wave.
"""

import os
import sys

sys.path.insert(0, "/opt/trn_rl_repo")

import numpy as np

N_CORES = 8
N, C_IN, H, W = 32, 128, 112, 112
C_OUT, KH, KW = 256, 3, 3
PER_CORE = N // N_CORES           # 4 images
NPAIR = H // 2                    # 56 row pairs
RPP = 4                           # row-pairs per psum half-tile
NPG = NPAIR // RPP                # 14 pair-groups
PG_HALF = NPG // 2                # 7 pair-groups per image half
MT = C_OUT // 128                 # 2 C_out tiles
WP = W + 2                        # padded columns: 114
HHALF = 28                        # row pairs per half-image
LDR = 58                          # staged rows per half (with halo+pad)

_CACHED = {}

# drop Ldweights instructions whose weights are already loaded (adjacent
# same-weight matmul pairs); 0 = self-loading matmuls as emitted
DEDUP_LDW = os.environ.get("CONV_DEDUP_LDW", "1") == "1"
# input-transform engine: gpsimd | dve | split
V_ENGINE = os.environ.get("CONV_V_ENGINE", "dve")
# inverse-transform staging: m0s (ACT stages M0 copy + M1+bias; frees PSUM
# banks earliest) | m1b (single ACT stage of M1+bias)
INV_SCHEME = os.environ.get("CONV_INV_SCHEME", "m1b")
# output-DMA queue: act (qACT hwdge) | sp (qSP hwdge)
OUT_Q = os.environ.get("CONV_OUT_Q", "act")
# matmul block j-emission order
J_ORDER = tuple(int(c) for c in os.environ.get("CONV_J_ORDER", "0123"))
# stage an f16 copy of the image (GPSIMD) so the DVE V-transform ops are
# all-16-bit and eligible for the DVE 2x perf mode
F16_LD = os.environ.get("CONV_F16_LD", "0") == "1"


def _dedup_ldweights(nc):
    """Remove back-to-back redundant PE weight loads.

    The tile scheduler splits every matmul into Ldweights + Matmult.  When
    consecutive matmuls share the same stationary operand (our s=0/s=1
    pg pair), the second Ldweights reloads identical weights; HW pays
    ~53ns each.  Drop the duplicate, preserving its sync waits by moving
    any non-duplicate waits onto the next instruction.
    """
    import concourse.mybir as mybir

    def wait_key(w):
        return (w.sync_type, w.id, w.wait_mode, w.wait_value, str(w.wait_reg))

    def ldw_key(inst):
        return (str(inst.ins[0]), str(inst.perf_mode), str(inst.is_transpose),
                str(inst.tile_position))

    n_removed = 0
    for fn in nc.m.functions:
        for blk in fn.blocks:
            insts = blk.instructions
            rev = {}
            for inst in insts:
                for dep_name, _info in inst.dependency_edges():
                    rev.setdefault(dep_name, []).append(inst)
            keep = []
            last = None          # (inst, key) of last kept Ldweights
            removed_here = set()
            for idx, inst in enumerate(insts):
                op = inst.opcode
                if getattr(inst, "engine", None) != mybir.EngineType.PE:
                    keep.append(inst)
                    continue
                if op == "Ldweights":
                    key = ldw_key(inst)
                    si = inst.sync_info
                    updates = list(si.on_update) if si else []
                    if last is not None and key == last[1] and not updates:
                        kept_si = last[0].sync_info
                        kept_waits = {wait_key(w) for w in
                                      (kept_si.on_wait if kept_si else [])}
                        waits = list(si.on_wait) if si else []
                        extra = [w for w in waits
                                 if wait_key(w) not in kept_waits]
                        if extra and idx + 1 < len(insts):
                            nxt = insts[idx + 1]
                            nsi = nxt.sync_info
                            if nsi is None:
                                nxt.sync_info = mybir.SyncInfo(
                                    on_wait=list(extra), on_update=[])
                            else:
                                nsi.on_wait = list(nsi.on_wait) + list(extra)
                        for dep_inst in rev.get(inst.name, []):
                            dep_inst.remap_dependency_names(
                                {inst.name: last[0].name})
                        removed_here.add(inst.name)
                        n_removed += 1
                        continue
                    last = (inst, key)
                    keep.append(inst)
                elif op in ("Matmult", "MatmultMx"):
                    if getattr(inst, "is_transpose", None):
                        last = None
                    keep.append(inst)
                else:
                    last = None
                    keep.append(inst)
            if removed_here:
                blk.instructions[:] = keep
    return n_removed


def _build(repeat: int = 1):
    import contextlib
    import concourse.tile as tile
    import concourse.mybir as mybir
    from concourse import bacc

    F32 = mybir.dt.float32
    F16 = mybir.dt.float16
    ADD = mybir.AluOpType.add
    SUB = mybir.AluOpType.subtract

    nc = bacc.Bacc("TRN2", target_bir_lowering=False, debug=False)
    img_d = nc.dram_tensor("img", [PER_CORE, C_IN, H, W], F32,
                           kind="ExternalInput").ap()
    # U: [C_in, j, kx, mt, 128] f16 (host-transformed weights)
    u_d = nc.dram_tensor("wt", [C_IN, 4, KW, MT, 128], F16,
                         kind="ExternalInput").ap()
    bias_d = nc.dram_tensor("bias", [128, MT], F32, kind="ExternalInput").ap()
    out_d = nc.dram_tensor("out", [PER_CORE, C_OUT, H, W], F32,
                           kind="ExternalOutput").ap()

    if V_ENGINE == "gpsimd":
        v_engines = ["g", "g", "g", "g"]
    elif V_ENGINE == "dve":
        v_engines = ["v", "v", "v", "v"]
    elif V_ENGINE == "split":
        v_engines = ["g", "g", "v", "v"]
    else:
        # per-j engine string, e.g. "vvvg"
        assert len(V_ENGINE) == 4 and set(V_ENGINE) <= {"g", "v"}, V_ENGINE
        v_engines = list(V_ENGINE)

    with tile.TileContext(nc) as tc:
        with tc.tile_pool(name="const", bufs=1) as const_pool, \
             tc.tile_pool(name="ld", bufs=2) as ld_pool, \
             tc.tile_pool(name="ldh", bufs=2) as ldh_pool, \
             tc.tile_pool(name="vbuf", bufs=3) as v_pool, \
             tc.tile_pool(name="tmp", bufs=6) as tmp_pool, \
             tc.tile_pool(name="outsb", bufs=3) as out_pool, \
             tc.tile_pool(name="psum", bufs=4, space="PSUM") as psum_pool:

            u_sb = const_pool.tile([C_IN, 4, KW, MT, 128], F16)
            nc.sync.dma_start(u_sb[:], u_d[:])
            bias_sb = const_pool.tile([128, MT], F32)
            nc.sync.dma_start(bias_sb[:], bias_d[:])

            loop_ctx = tc.For_i(0, repeat, 1) if repeat > 1 \
                else contextlib.nullcontext()
            with loop_ctx:
              for i in range(PER_CORE):
                # ---- load + input transform, one half-image per step ----
                vhalves = []
                for h in range(2):
                    ld = ld_pool.tile([C_IN, LDR, W], F32)
                    # two DMA chunks per half: shorter fabric occupancy per
                    # transfer so output DMAs interleave with less latency
                    if h == 0:
                        # slots 0..57 = img rows -1..56 (row -1 zero)
                        nc.vector.memset(ld[:, 0, :], 0.0)
                        nc.sync.dma_start(ld[:, 1:30, :], img_d[i, :, 0:29, :])
                        nc.sync.dma_start(ld[:, 30:58, :],
                                          img_d[i, :, 29:57, :])
                    else:
                        # slots 0..57 = img rows 55..112 (row 112 zero)
                        nc.sync.dma_start(ld[:, 0:29, :],
                                          img_d[i, :, 55:84, :])
                        nc.sync.dma_start(ld[:, 29:57, :],
                                          img_d[i, :, 84:112, :])
                        nc.vector.memset(ld[:, 57, :], 0.0)

                    v = v_pool.tile([C_IN, 4, HHALF, WP], F16)
                    nc.vector.memset(v[:, :, :, 0], 0.0)
                    nc.vector.memset(v[:, :, :, WP - 1], 0.0)
                    if F16_LD:
                        # f16 staging copy on the idle GPSIMD: V-transform
                        # inputs/outputs all 16-bit -> DVE 2x mode
                        ldh = ldh_pool.tile([C_IN, LDR, W], F16)
                        nc.gpsimd.tensor_copy(out=ldh[:], in_=ld[:])
                        dsrc = ldh
                    else:
                        dsrc = ld
                    d0 = dsrc[:, 0:55:2, :]
                    d1 = dsrc[:, 1:56:2, :]
                    d2 = dsrc[:, 2:57:2, :]
                    d3 = dsrc[:, 3:58:2, :]
                    vops = [(0, d0, d2, SUB), (1, d1, d2, ADD),
                            (2, d2, d1, SUB), (3, d1, d3, SUB)]
                    for j, a, b, op in vops:
                        eng = nc.gpsimd if v_engines[j] == "g" else nc.vector
                        eng.tensor_tensor(v[:, j, :, 1:1 + W], a, b, op)
                    vhalves.append(v)

                # ---- matmuls + inverse transform per pgpair/mt ----
                for q in range(0, NPG, 2):
                    for mt in range(MT):
                        ps = []
                        for j in range(4):
                            pj = psum_pool.tile([128, 2, 512], F32,
                                                name=f"ps{j}", tag="ps")
                            ps.append(pj)
                        # j order (1,0,2,3): M1 finishes first so the
                        # ACT m1b stage + DVE chain frees every bank >=1.2us
                        # before the next block's matmuls need it
                        for j in J_ORDER:
                            for kx in range(KW):
                                for s in range(2):
                                    pg = q + s
                                    hh, lp = divmod(pg, PG_HALF)
                                    nc.tensor.matmul(
                                        ps[j][:, s, 0:448],
                                        lhsT=u_sb[:, j, kx, mt, :],
                                        rhs=vhalves[hh][:, j,
                                                        4 * lp:4 * lp + RPP,
                                                        kx:kx + W],
                                        start=(kx == 0), stop=(kx == KW - 1),
                                    )
                        # [128,2,448] -> [128,2,4,112] views of M_j
                        mv = [ps[j][:, :, 0:448].rearrange(
                                  "p s (a w) -> p s a w", a=RPP)
                              for j in range(4)]
                        osb = out_pool.tile([128, 2, RPP, 2, W], F32)
                        if INV_SCHEME != "skel":
                            m1b = tmp_pool.tile([128, 2, RPP, W], F32,
                                                tag="tmp")
                            a0 = tmp_pool.tile([128, 2, RPP, W], F32,
                                               tag="tmp")
                            t1 = tmp_pool.tile([128, 2, RPP, W], F32,
                                               tag="tmp")
                        if INV_SCHEME == "spread":
                            # ACT stages M1+b and M2; SBUF-only combines on
                            # GPSIMD; DVE only does the two PSUM-read ops
                            m2s = tmp_pool.tile([128, 2, RPP, W], F32,
                                                tag="tmp")
                            nc.scalar.add(m1b[:], mv[1],
                                          bias_sb[:, mt:mt + 1])
                            nc.scalar.copy(m2s[:], mv[2])
                            nc.vector.tensor_tensor(a0[:], m1b[:], mv[0],
                                                    ADD)
                            nc.gpsimd.tensor_tensor(osb[:, :, :, 0, :],
                                                    a0[:], m2s[:], ADD)
                            nc.gpsimd.tensor_tensor(t1[:], m1b[:], m2s[:],
                                                    SUB)
                            nc.vector.tensor_tensor(osb[:, :, :, 1, :],
                                                    t1[:], mv[3], SUB)
                        elif INV_SCHEME == "skel":
                            # DIAGNOSTIC: no DVE inverse; ACT copies M1 pair
                            # into both parities (wrong results, times the
                            # PE + DMA + ACT skeleton)
                            nc.scalar.copy(osb[:, :, :, 0, :], mv[1])
                            nc.scalar.copy(osb[:, :, :, 1, :], mv[2])
                        elif INV_SCHEME == "m0s":
                            # ACT drains M0 (then M1) right as their banks
                            # finish accumulating -> earliest bank reuse
                            m0s = tmp_pool.tile([128, 2, RPP, W], F32, tag="tmp")
                            nc.scalar.copy(m0s[:], mv[0])
                            nc.scalar.add(m1b[:], mv[1],
                                          bias_sb[:, mt:mt + 1])
                            nc.vector.tensor_tensor(a0[:], m0s[:], m1b[:],
                                                    ADD)
                        else:
                            nc.scalar.add(m1b[:], mv[1],
                                          bias_sb[:, mt:mt + 1])
                            nc.vector.tensor_tensor(a0[:], m1b[:], mv[0],
                                                    ADD)
                        if INV_SCHEME not in ("skel", "spread"):
                            nc.vector.tensor_tensor(osb[:, :, :, 0, :],
                                                    a0[:], mv[2], ADD)
                            nc.vector.tensor_tensor(t1[:], m1b[:], mv[2],
                                                    SUB)
                            nc.vector.tensor_tensor(osb[:, :, :, 1, :],
                                                    t1[:], mv[3], SUB)
                        for s in range(2):
                            # qACT hwdge queue: keeps the big output stream
                            # off qSP, which carries the input loads
                            out_eng = nc.scalar if OUT_Q == "act" \
                                else nc.sync
                            out_eng.dma_start(
                                out_d[i, mt * 128:(mt + 1) * 128,
                                      8 * (q + s):8 * (q + s) + 8, :],
                                osb[:, s, :, :, :])

    if DEDUP_LDW:
        _dedup_ldweights(nc)
    nc.compile()
    return nc


def _make_runner(nc, donate=True):
    """Cached sharded-jit runner for `nc` on 8 cores (see kernel.py)."""
    import jax
    from jax.sharding import Mesh, PartitionSpec, NamedSharding
    from jax.experimental.shard_map import shard_map
    import concourse.mybir as mybir
    from concourse import bass2jax

    bass2jax.install_neuronx_cc_hook()

    partition_name = nc.partition_id_tensor.name if nc.partition_id_tensor else None
    in_names, out_names, out_avals, zero_outs = [], [], [], []
    for alloc in nc.m.functions[0].allocations:
        if not isinstance(alloc, mybir.MemoryLocationSet):
            continue
        name = alloc.memorylocations[0].name
        if alloc.kind == "ExternalInput":
            if name != partition_name:
                in_names.append(name)
        elif alloc.kind == "ExternalOutput":
            shape = tuple(alloc.tensor_shape)
            dtype = mybir.dt.np(alloc.dtype)
            out_names.append(name)
            out_avals.append(jax.core.ShapedArray(shape, dtype))
            zero_outs.append(np.zeros(shape, dtype))
    n_params = len(in_names)
    n_outs = len(out_avals)
    all_in_names = list(in_names) + list(out_names)
    if partition_name is not None:
        all_in_names.append(partition_name)

    def _body(*args):
        operands = list(args)
        if partition_name is not None:
            operands.append(bass2jax.partition_id_tensor())
        outs = bass2jax._bass_exec_p.bind(
            *operands,
            out_avals=tuple(out_avals),
            in_names=tuple(all_in_names),
            out_names=tuple(out_names),
            lowering_input_output_aliases=(),
            sim_require_finite=True,
            sim_require_nnan=True,
            nc=nc,
        )
        return tuple(outs)

    devices = jax.devices()[:N_CORES]
    mesh = Mesh(np.asarray(devices), ("core",))
    in_specs = (PartitionSpec("core"),) * (n_params + n_outs)
    out_specs = (PartitionSpec("core"),) * len(out_names)
    kwargs = dict(keep_unused=True)
    if donate:
        kwargs["donate_argnums"] = tuple(range(n_params, n_params + n_outs))
    sharded = jax.jit(
        shard_map(_body, mesh=mesh, in_specs=in_specs, out_specs=out_specs,
                  check_rep=False),
        **kwargs)
    sharding = NamedSharding(mesh, PartitionSpec("core"))

    def prep(in_maps, device_put=False):
        concat = [np.concatenate([np.asarray(m[name]) for m in in_maps], axis=0)
                  for name in in_names]
        concat += [np.concatenate([z] * N_CORES, axis=0) for z in zero_outs]
        if device_put:
            concat = [jax.device_put(a, sharding) for a in concat]
        return concat

    def run(args):
        return sharded(*args)

    def to_results(outs):
        results = [dict() for _ in range(N_CORES)]
        for name, arr in zip(out_names, outs):
            arr = np.asarray(arr)
            per = np.split(arr, N_CORES, axis=0)
            for c in range(N_CORES):
                results[c][name] = per[c]
        return results

    return prep, run, to_results


def _host_weights(weight: np.ndarray) -> np.ndarray:
    """[C_out, C_in, 3, 3] f32 -> U [C_in, j(4), kx(3), mt(2), 128] f16."""
    w = weight.astype(np.float64)
    g0 = w[:, :, 0, :]
    g1 = w[:, :, 1, :]
    g2 = w[:, :, 2, :]
    u = np.stack([g0, (g0 + g1 + g2) * 0.5, (g0 - g1 + g2) * 0.5, g2],
                 axis=0)                      # [j, C_out, C_in, kx]
    u = u.transpose(2, 0, 3, 1)               # [C_in, j, kx, C_out]
    u = u.reshape(C_IN, 4, KW, MT, 128)
    return np.ascontiguousarray(u.astype(np.float16))


def kernel(img: np.ndarray, weight: np.ndarray, bias: np.ndarray) -> np.ndarray:
    img = np.ascontiguousarray(np.asarray(img, dtype=np.float32))
    weight = np.ascontiguousarray(np.asarray(weight, dtype=np.float32))
    bias = np.ascontiguousarray(np.asarray(bias, dtype=np.float32))

    u = _host_weights(weight)
    bias2 = np.ascontiguousarray(bias.reshape(MT, 128).T)

    if "nc" not in _CACHED:
        _CACHED["nc"] = _build()
        _CACHED["runner"] = _make_runner(_CACHED["nc"], donate=False)
    prep, run, to_results = _CACHED["runner"]

    shards = img.reshape(N_CORES, PER_CORE, C_IN, H, W)
    in_maps = [{"img": shards[i], "wt": u, "bias": bias2}
               for i in range(N_CORES)]

    outs = run(prep(in_maps))
    results = to_results(outs)
    _CACHED["last_results"] = results
    return np.concatenate([r["out"] for r in results], axis=0)
